# revision 8
# baseline (speedup 1.0000x reference)
"""Trainium2 Bass kernel for the AdaptiveSNN problem.

Strategy (pure data parallelism across 8 NeuronCores, batch 16384 -> 2048/core):
  - host: transpose x/W1 so the contraction dim (784) lands on SBUF partitions
  - cur1 = W1 @ x^T + b1 computed once per core in fp32 on the TensorEngine
    (layout [128 neurons, 2048 batch])
  - 25 sequential LIF steps; the whole membrane update
        m' = (beta*m + cur) - (m > 1)
    is ONE custom DVE instruction (registered below). Layer-1 spikes for the
    layer-2 matmul come from the ScalarEngine as Sign(m-1) in bf16 (+-1 exact).
  - layer-2 matmul per 128-batch chunk: stationary = sign chunk (bf16),
    moving = 0.5*W2^T split hi/lo in bf16 (exact to ~1e-7), plus a K=1
    fp32 ones x cc matmul that adds the constant 0.5*sum(W2)+b2 row.
    Output lands batch-major [128, 16*10] so layer-2 elementwise is cheap.
  - outputs DMA'd per step as [25, 128, 160] per core; host reassembles.
"""
import numpy as np
import ml_dtypes

import concourse.bacc as bacc
import concourse.mybir as mybir
import concourse.tile as tile
import concourse.dve_ops as dve_ops
from concourse.dve_spec import Spec, Src0, Src1, C0, C1
from concourse.dve_ops import DveOp
from concourse.bass_utils import run_bass_kernel_spmd

F32 = mybir.dt.float32
BF16 = mybir.dt.bfloat16

N_CORES = 8
B_FULL = 16384
B = B_FULL // N_CORES          # 2048 batch rows per core
D_IN = 784                     # 28*28
H1 = 128
H2 = 10
STEPS = 25
KT = 112                       # K-tile size: 784 = 7 * 112
NKT = D_IN // KT
NCHUNK = B // 128              # 16 batch chunks per core
THRESH = 1.0


def _register_lif():
    """Custom DVE op: out = ((in0*s0 + in1) - (in0 > s1)).

    Matches the reference's fp32 association order exactly:
    new_mem = (beta*mem + cur) - reset.
    """
    if "LIF_STEP_ANT" in dve_ops._SUB_OPCODE_FOR_NAME:
        return next(op for op in dve_ops.OPS if op.name == "LIF_STEP_ANT")
    op = DveOp(
        "LIF_STEP_ANT",
        Spec(
            body=(Src0 * C0 + Src1) - (Src0 > C1),
            reference=lambda in0, in1, s0, s1, imm2: (
                (in0 * s0 + in1) - (in0 > s1).astype(np.float32)
            ),
        ),
        subdim=False,
        uops_sha={"v3": "4d971942aba05d49", "v4": "da6677450a1cb1b9"},
    )
    dve_ops.OPS.append(op)
    dve_ops._SUB_OPCODE_FOR_NAME[op.name] = (
        dve_ops._CUSTOM_DVE_ROW_BASE + len(dve_ops.OPS) - 1
    )
    dve_ops.CUSTOM_DVE_SPECS[op.name] = op.spec
    return op


_GRAPH_CACHE = {}


def _build_graph(beta1: float, beta2: float):
    key = (beta1, beta2)
    if key in _GRAPH_CACHE:
        return _GRAPH_CACHE[key]
    LIF = _register_lif()
    Sign = mybir.ActivationFunctionType.Sign

    nc = bacc.Bacc("TRN2", target_bir_lowering=False, debug=False,
                   num_devices=N_CORES)

    xt_d = nc.dram_tensor("xt", [D_IN, B], F32, kind="ExternalInput").ap()
    w1t_d = nc.dram_tensor("w1t", [D_IN, H1], F32, kind="ExternalInput").ap()
    b1_d = nc.dram_tensor("b1", [H1, 1], F32, kind="ExternalInput").ap()
    w2h_d = nc.dram_tensor("w2h", [H1, H2], BF16, kind="ExternalInput").ap()
    w2l_d = nc.dram_tensor("w2l", [H1, H2], BF16, kind="ExternalInput").ap()
    cc_d = nc.dram_tensor("cc160", [128, NCHUNK * H2], F32, kind="ExternalInput").ap()

    out_spk = nc.dram_tensor("out_spk", [STEPS, 128, NCHUNK * H2], F32,
                             kind="ExternalOutput").ap()
    out_mem = nc.dram_tensor("out_mem", [STEPS, 128, NCHUNK * H2], F32,
                             kind="ExternalOutput").ap()

    with tile.TileContext(nc) as tc:
        with tc.tile_pool(name="const", bufs=1) as cpool, \
             tc.tile_pool(name="xin", bufs=1) as xpool, \
             tc.tile_pool(name="m1p", bufs=2) as m1pool, \
             tc.tile_pool(name="m2p", bufs=3) as m2pool, \
             tc.tile_pool(name="s2p", bufs=3) as s2pool, \
             tc.tile_pool(name="sgp", bufs=2) as sgpool, \
             tc.tile_pool(name="ps1", bufs=1, space="PSUM") as ps1pool, \
             tc.tile_pool(name="ps2", bufs=2, space="PSUM") as ps2pool:

            # ---- load constants and input shard ----
            w1t_tiles = []
            xt_tiles = []
            for k in range(NKT):
                wt = cpool.tile([KT, H1], F32, tag=f"w1t{k}")
                nc.sync.dma_start(wt[:], w1t_d[k * KT:(k + 1) * KT, :])
                w1t_tiles.append(wt)
            for k in range(NKT):
                xt = xpool.tile([KT, B], F32, tag=f"xt{k}")
                half = B // 2
                nc.sync.dma_start(xt[:, :half], xt_d[k * KT:(k + 1) * KT, :half])
                nc.sync.dma_start(xt[:, half:], xt_d[k * KT:(k + 1) * KT, half:])
                xt_tiles.append(xt)
            b1_t = cpool.tile([H1, 1], F32, tag="b1")
            nc.sync.dma_start(b1_t[:], b1_d)
            w2h_t = cpool.tile([H1, H2], BF16, tag="w2h")
            nc.sync.dma_start(w2h_t[:], w2h_d)
            w2l_t = cpool.tile([H1, H2], BF16, tag="w2l")
            nc.sync.dma_start(w2l_t[:], w2l_d)
            cc_t = cpool.tile([128, NCHUNK * H2], F32, tag="cc160")
            nc.sync.dma_start(cc_t[:], cc_d)
            neg1_t = cpool.tile([H1, 1], F32, tag="neg1")
            nc.vector.memset(neg1_t[:], -1.0)

            # ---- cur1 = W1 @ x^T + b1 : [128 neurons, 2048 batch] ----
            cur1_t = cpool.tile([H1, B], F32, tag="cur1")
            NT = B // 512
            ps1_tiles = [ps1pool.tile([H1, 512], F32, tag=f"ps1_{nt}",
                                      name=f"ps1_{nt}") for nt in range(NT)]
            for k in range(NKT):
                for nt in range(NT):
                    nc.tensor.matmul(
                        ps1_tiles[nt][:], w1t_tiles[k][:],
                        xt_tiles[k][:, nt * 512:(nt + 1) * 512],
                        start=(k == 0), stop=(k == NKT - 1))
            for nt in range(NT):
                # psum -> sbuf with +b1 per-partition bias on the ScalarEngine
                nc.scalar.add(cur1_t[:, nt * 512:(nt + 1) * 512],
                              ps1_tiles[nt][:], b1_t[:])

            # ---- states ----
            m1 = m1pool.tile([H1, B], F32, tag="m1")
            nc.vector.memset(m1[:], 0.0)
            m2 = m2pool.tile([128, NCHUNK * H2], F32, tag="m2")
            nc.vector.memset(m2[:], 0.0)
            r2 = s2pool.tile([128, NCHUNK * H2], F32, tag="s2")
            nc.vector.memset(r2[:], 0.0)

            # ---- time loop ----
            # Software-pipelined by one step: step t's layer-2 DVE work is
            # emitted AFTER step t+1's layer-1 LIF, so the DVE never stalls
            # waiting for the ScalarE(sign) -> TensorE(matmul) chain.
            def finish_l2(ps2, t, m2_cur, r_cur):
                # cur2 = psum + cc  (only step that must read PSUM -> DVE)
                cur2 = s2pool.tile([128, NCHUNK * H2], F32, tag="cur2")
                nc.vector.affine_then_add(cur2[:], ps2[:], cc_t[:], 1.0, 0.0)
                # membrane update on DVE (one fused op); spike on GpSimd
                m2n = m2pool.tile([128, NCHUNK * H2], F32, tag="m2")
                nc.vector._custom_dve(LIF, out=m2n[:], in0=m2_cur[:],
                                      in1=cur2[:], s0=beta2, s1=THRESH)
                r_new = s2pool.tile([128, NCHUNK * H2], F32, tag="s2")
                nc.gpsimd.tensor_scalar(r_new[:], m2n[:], THRESH, None,
                                        mybir.AluOpType.is_gt)
                nc.sync.dma_start(out_mem[t], m2n[:])
                nc.sync.dma_start(out_spk[t], r_new[:])
                return m2n, r_new

            pend = None  # (psum2, t) of the step whose layer-2 is unfinished
            for t in range(STEPS):
                # layer-1 membrane update: one DVE instruction
                m1n = m1pool.tile([H1, B], F32, tag="m1")
                nc.vector._custom_dve(LIF, out=m1n[:], in0=m1[:], in1=cur1_t[:],
                                      s0=beta1, s1=THRESH)
                m1 = m1n
                if pend is not None:
                    m2, r2 = finish_l2(pend[0], pend[1], m2, r2)
                # spikes as signs (bf16, +-1) on the ScalarEngine
                sg = sgpool.tile([H1, B], BF16, tag="sg")
                nc.scalar.activation(sg[:], m1[:], Sign, bias=neg1_t[:])
                # layer-2: cur2 = cc + 0.5*W2@sg  (batch-major [128, 160])
                ps2 = ps2pool.tile([128, NCHUNK * H2], F32, tag="ps2")
                for c in range(NCHUNK):
                    o = ps2[:, c * H2:(c + 1) * H2]
                    sgc = sg[:, c * 128:(c + 1) * 128]
                    nc.tensor.matmul(o, sgc, w2h_t[:], start=True, stop=False)
                    nc.tensor.matmul(o, sgc, w2l_t[:], start=False, stop=True)
                pend = (ps2, t)
            m2, r2 = finish_l2(pend[0], pend[1], m2, r2)

    nc.compile()
    _GRAPH_CACHE[key] = nc
    return nc


def prepare_in_maps(x, W1, b1, W2, b2):
    x = np.asarray(x, dtype=np.float32)
    W1 = np.asarray(W1, dtype=np.float32)
    b1 = np.asarray(b1, dtype=np.float32)
    W2 = np.asarray(W2, dtype=np.float32)
    b2 = np.asarray(b2, dtype=np.float32)
    xf = x.reshape(B_FULL, D_IN)
    xT = np.ascontiguousarray(xf.T)                       # [784, 16384]
    W1T = np.ascontiguousarray(W1.T)                      # [784, 128]
    b1c = np.ascontiguousarray(b1.reshape(H1, 1))
    W2T_half = 0.5 * W2.T                                 # [128, 10]
    w2h = W2T_half.astype(ml_dtypes.bfloat16)
    w2l = (W2T_half - w2h.astype(np.float32)).astype(ml_dtypes.bfloat16)
    ccrow = (0.5 * W2.sum(axis=1) + b2).astype(np.float32)
    cc160 = np.ascontiguousarray(np.broadcast_to(np.tile(ccrow, NCHUNK), (128, NCHUNK * H2)).astype(np.float32))
    in_maps = []
    for i in range(N_CORES):
        shard = np.ascontiguousarray(xT[:, i * B:(i + 1) * B])
        in_maps.append({
            "xt": shard, "w1t": W1T, "b1": b1c,
            "w2h": w2h, "w2l": w2l, "cc160": cc160,
        })
    return in_maps


def kernel(x, W1, b1, W2, b2, beta1, beta2):
    bb1 = float(np.clip(np.float32(beta1), 0.0, 1.0))
    bb2 = float(np.clip(np.float32(beta2), 0.0, 1.0))
    in_maps = prepare_in_maps(x, W1, b1, W2, b2)
    nc = _build_graph(bb1, bb2)
    res = run_bass_kernel_spmd(nc, in_maps, list(range(N_CORES)), trace=False)

    spk_parts, mem_parts = [], []
    for i in range(N_CORES):
        r = res.results[i]
        # [25, 128, 16*10] -> [25, 2048, 10]: batch = chunk*128 + partition
        spk = r["out_spk"].reshape(STEPS, 128, NCHUNK, H2)
        mem = r["out_mem"].reshape(STEPS, 128, NCHUNK, H2)
        spk_parts.append(np.transpose(spk, (0, 2, 1, 3)).reshape(STEPS, B, H2))
        mem_parts.append(np.transpose(mem, (0, 2, 1, 3)).reshape(STEPS, B, H2))
    spk2 = np.concatenate(spk_parts, axis=1).astype(np.float32)
    mem2 = np.concatenate(mem_parts, axis=1).astype(np.float32)
    return spk2, mem2


# revision 10
# speedup vs baseline: 1.1133x; 1.1133x over previous
"""Trainium2 Bass kernel for the AdaptiveSNN problem.

Strategy (pure data parallelism across 8 NeuronCores, batch 16384 -> 2048/core):
  - host: transpose x/W1 so the contraction dim (784) lands on SBUF partitions
  - cur1 = W1 @ x^T + b1 computed once per core in fp32 on the TensorEngine
    (layout [128 neurons, 2048 batch])
  - 25 sequential LIF steps; the whole membrane update
        m' = (beta*m + cur) - (m > 1)
    is ONE custom DVE instruction (registered below). Layer-1 spikes for the
    layer-2 matmul come from the ScalarEngine as Sign(m-1) in bf16 (+-1 exact).
  - layer-2 matmul per 128-batch chunk: stationary = sign chunk (bf16),
    moving = 0.5*W2^T split hi/lo in bf16 (exact to ~1e-7), plus a K=1
    fp32 ones x cc matmul that adds the constant 0.5*sum(W2)+b2 row.
    Output lands batch-major [128, 16*10] so layer-2 elementwise is cheap.
  - outputs DMA'd per step as [25, 128, 160] per core; host reassembles.
"""
import numpy as np
import ml_dtypes

import concourse.bacc as bacc
import concourse.mybir as mybir
import concourse.tile as tile
import concourse.dve_ops as dve_ops
from concourse.dve_spec import Spec, Src0, Src1, C0, C1
from concourse.dve_ops import DveOp
from concourse.bass_utils import run_bass_kernel_spmd

F32 = mybir.dt.float32
BF16 = mybir.dt.bfloat16

N_CORES = 8
B_FULL = 16384
B = B_FULL // N_CORES          # 2048 batch rows per core
D_IN = 784                     # 28*28
H1 = 128
H2 = 10
STEPS = 25
KT = 112                       # K-tile size: 784 = 7 * 112
NKT = D_IN // KT
NCHUNK = B // 128              # 16 batch chunks per core
THRESH = 1.0


def _register_lif():
    """Custom DVE op: out = ((in0*s0 + in1) - (in0 > s1)).

    Matches the reference's fp32 association order exactly:
    new_mem = (beta*mem + cur) - reset.
    """
    if "LIF_STEP_ANT" in dve_ops._SUB_OPCODE_FOR_NAME:
        return next(op for op in dve_ops.OPS if op.name == "LIF_STEP_ANT")
    op = DveOp(
        "LIF_STEP_ANT",
        Spec(
            body=(Src0 * C0 + Src1) - (Src0 > C1),
            reference=lambda in0, in1, s0, s1, imm2: (
                (in0 * s0 + in1) - (in0 > s1).astype(np.float32)
            ),
        ),
        subdim=False,
        uops_sha={"v3": "4d971942aba05d49", "v4": "da6677450a1cb1b9"},
    )
    dve_ops.OPS.append(op)
    dve_ops._SUB_OPCODE_FOR_NAME[op.name] = (
        dve_ops._CUSTOM_DVE_ROW_BASE + len(dve_ops.OPS) - 1
    )
    dve_ops.CUSTOM_DVE_SPECS[op.name] = op.spec
    return op


_GRAPH_CACHE = {}


def _build_graph(beta1: float, beta2: float):
    key = (beta1, beta2)
    if key in _GRAPH_CACHE:
        return _GRAPH_CACHE[key]
    LIF = _register_lif()
    Sign = mybir.ActivationFunctionType.Sign

    nc = bacc.Bacc("TRN2", target_bir_lowering=False, debug=False,
                   num_devices=N_CORES)

    xt_d = nc.dram_tensor("xt", [D_IN, B], F32, kind="ExternalInput").ap()
    w1t_d = nc.dram_tensor("w1t", [D_IN, H1], F32, kind="ExternalInput").ap()
    b1_d = nc.dram_tensor("b1", [H1, 1], F32, kind="ExternalInput").ap()
    w2h_d = nc.dram_tensor("w2h", [H1, H2], BF16, kind="ExternalInput").ap()
    w2l_d = nc.dram_tensor("w2l", [H1, H2], BF16, kind="ExternalInput").ap()
    cc_d = nc.dram_tensor("cc160", [128, NCHUNK * H2], F32, kind="ExternalInput").ap()

    out_spk = nc.dram_tensor("out_spk", [STEPS, 128, NCHUNK * H2], F32,
                             kind="ExternalOutput").ap()
    out_mem = nc.dram_tensor("out_mem", [STEPS, 128, NCHUNK * H2], F32,
                             kind="ExternalOutput").ap()

    with tile.TileContext(nc) as tc:
        with tc.tile_pool(name="const", bufs=1) as cpool, \
             tc.tile_pool(name="xin", bufs=1) as xpool, \
             tc.tile_pool(name="m1p", bufs=2) as m1pool, \
             tc.tile_pool(name="m2p", bufs=3) as m2pool, \
             tc.tile_pool(name="s2p", bufs=3) as s2pool, \
             tc.tile_pool(name="sgp", bufs=2) as sgpool, \
             tc.tile_pool(name="ps1", bufs=1, space="PSUM") as ps1pool, \
             tc.tile_pool(name="ps2", bufs=2, space="PSUM") as ps2pool:

            # ---- load constants and input shard ----
            w1t_tiles = []
            xt_tiles = []
            for k in range(NKT):
                wt = cpool.tile([KT, H1], F32, tag=f"w1t{k}")
                nc.sync.dma_start(wt[:], w1t_d[k * KT:(k + 1) * KT, :])
                w1t_tiles.append(wt)
            dma_engines = [nc.sync, nc.scalar, nc.gpsimd]
            for k in range(NKT):
                xt = xpool.tile([KT, B], F32, tag=f"xt{k}")
                half = B // 2
                e0 = dma_engines[(2 * k) % len(dma_engines)]
                e1 = dma_engines[(2 * k + 1) % len(dma_engines)]
                e0.dma_start(xt[:, :half], xt_d[k * KT:(k + 1) * KT, :half])
                e1.dma_start(xt[:, half:], xt_d[k * KT:(k + 1) * KT, half:])
                xt_tiles.append(xt)
            b1_t = cpool.tile([H1, 1], F32, tag="b1")
            nc.sync.dma_start(b1_t[:], b1_d)
            w2h_t = cpool.tile([H1, H2], BF16, tag="w2h")
            nc.sync.dma_start(w2h_t[:], w2h_d)
            w2l_t = cpool.tile([H1, H2], BF16, tag="w2l")
            nc.sync.dma_start(w2l_t[:], w2l_d)
            cc_t = cpool.tile([128, NCHUNK * H2], F32, tag="cc160")
            nc.sync.dma_start(cc_t[:], cc_d)
            neg1_t = cpool.tile([H1, 1], F32, tag="neg1")
            nc.vector.memset(neg1_t[:], -1.0)
            warm_t = cpool.tile([H1, 1], F32, tag="warm")
            nc.scalar.activation(warm_t[:], neg1_t[:], Sign, bias=neg1_t[:])

            # ---- cur1 = W1 @ x^T + b1 : [128 neurons, 2048 batch] ----
            cur1_t = cpool.tile([H1, B], F32, tag="cur1")
            NT = B // 512
            ps1_tiles = [ps1pool.tile([H1, 512], F32, tag=f"ps1_{nt}",
                                      name=f"ps1_{nt}") for nt in range(NT)]
            for k in range(NKT):
                for nt in range(NT):
                    nc.tensor.matmul(
                        ps1_tiles[nt][:], w1t_tiles[k][:],
                        xt_tiles[k][:, nt * 512:(nt + 1) * 512],
                        start=(k == 0), stop=(k == NKT - 1))
            for nt in range(NT):
                # psum -> sbuf with +b1 per-partition bias on the ScalarEngine
                nc.scalar.add(cur1_t[:, nt * 512:(nt + 1) * 512],
                              ps1_tiles[nt][:], b1_t[:])

            # ---- states ----
            m1 = m1pool.tile([H1, B], F32, tag="m1")
            nc.vector.memset(m1[:], 0.0)
            m2 = m2pool.tile([128, NCHUNK * H2], F32, tag="m2")
            nc.vector.memset(m2[:], 0.0)
            r2 = s2pool.tile([128, NCHUNK * H2], F32, tag="s2")
            nc.vector.memset(r2[:], 0.0)

            # ---- time loop ----
            # Software-pipelined by one step: step t's layer-2 DVE work is
            # emitted AFTER step t+1's layer-1 LIF, so the DVE never stalls
            # waiting for the ScalarE(sign) -> TensorE(matmul) chain.
            def finish_l2(ps2, t, m2_cur, r_cur):
                # cur2 = psum + cc  (only step that must read PSUM -> DVE)
                cur2 = s2pool.tile([128, NCHUNK * H2], F32, tag="cur2")
                nc.vector.affine_then_add(cur2[:], ps2[:], cc_t[:], 1.0, 0.0)
                # membrane update on DVE (one fused op); spike on GpSimd
                m2n = m2pool.tile([128, NCHUNK * H2], F32, tag="m2")
                nc.vector._custom_dve(LIF, out=m2n[:], in0=m2_cur[:],
                                      in1=cur2[:], s0=beta2, s1=THRESH)
                g2 = s2pool.tile([128, NCHUNK * H2], F32, tag="g2")
                nc.scalar.activation(g2[:], m2n[:], Sign, bias=neg1_t[:])
                r_new = s2pool.tile([128, NCHUNK * H2], F32, tag="s2")
                nc.scalar.activation(r_new[:], g2[:],
                                     mybir.ActivationFunctionType.Relu)
                nc.sync.dma_start(out_mem[t], m2n[:])
                nc.sync.dma_start(out_spk[t], r_new[:])
                return m2n, r_new

            pend = None  # (psum2, t) of the step whose layer-2 is unfinished
            for t in range(STEPS):
                # layer-1 membrane update: one DVE instruction
                m1n = m1pool.tile([H1, B], F32, tag="m1")
                nc.vector._custom_dve(LIF, out=m1n[:], in0=m1[:], in1=cur1_t[:],
                                      s0=beta1, s1=THRESH)
                m1 = m1n
                if pend is not None:
                    m2, r2 = finish_l2(pend[0], pend[1], m2, r2)
                # spikes as signs (bf16, +-1) on the ScalarEngine
                sg = sgpool.tile([H1, B], BF16, tag="sg")
                nc.scalar.activation(sg[:], m1[:], Sign, bias=neg1_t[:])
                # layer-2: cur2 = cc + 0.5*W2@sg  (batch-major [128, 160])
                ps2 = ps2pool.tile([128, NCHUNK * H2], F32, tag="ps2")
                for c in range(NCHUNK):
                    o = ps2[:, c * H2:(c + 1) * H2]
                    sgc = sg[:, c * 128:(c + 1) * 128]
                    nc.tensor.matmul(o, sgc, w2h_t[:], start=True, stop=False)
                    nc.tensor.matmul(o, sgc, w2l_t[:], start=False, stop=True)
                pend = (ps2, t)
            m2, r2 = finish_l2(pend[0], pend[1], m2, r2)

    nc.compile()
    _GRAPH_CACHE[key] = nc
    return nc


def prepare_in_maps(x, W1, b1, W2, b2):
    x = np.asarray(x, dtype=np.float32)
    W1 = np.asarray(W1, dtype=np.float32)
    b1 = np.asarray(b1, dtype=np.float32)
    W2 = np.asarray(W2, dtype=np.float32)
    b2 = np.asarray(b2, dtype=np.float32)
    xf = x.reshape(B_FULL, D_IN)
    xT = np.ascontiguousarray(xf.T)                       # [784, 16384]
    W1T = np.ascontiguousarray(W1.T)                      # [784, 128]
    b1c = np.ascontiguousarray(b1.reshape(H1, 1))
    W2T_half = 0.5 * W2.T                                 # [128, 10]
    w2h = W2T_half.astype(ml_dtypes.bfloat16)
    w2l = (W2T_half - w2h.astype(np.float32)).astype(ml_dtypes.bfloat16)
    ccrow = (0.5 * W2.sum(axis=1) + b2).astype(np.float32)
    cc160 = np.ascontiguousarray(np.broadcast_to(np.tile(ccrow, NCHUNK), (128, NCHUNK * H2)).astype(np.float32))
    in_maps = []
    for i in range(N_CORES):
        shard = np.ascontiguousarray(xT[:, i * B:(i + 1) * B])
        in_maps.append({
            "xt": shard, "w1t": W1T, "b1": b1c,
            "w2h": w2h, "w2l": w2l, "cc160": cc160,
        })
    return in_maps


def kernel(x, W1, b1, W2, b2, beta1, beta2):
    bb1 = float(np.clip(np.float32(beta1), 0.0, 1.0))
    bb2 = float(np.clip(np.float32(beta2), 0.0, 1.0))
    in_maps = prepare_in_maps(x, W1, b1, W2, b2)
    nc = _build_graph(bb1, bb2)
    res = run_bass_kernel_spmd(nc, in_maps, list(range(N_CORES)), trace=False)

    spk_parts, mem_parts = [], []
    for i in range(N_CORES):
        r = res.results[i]
        # [25, 128, 16*10] -> [25, 2048, 10]: batch = chunk*128 + partition
        spk = r["out_spk"].reshape(STEPS, 128, NCHUNK, H2)
        mem = r["out_mem"].reshape(STEPS, 128, NCHUNK, H2)
        spk_parts.append(np.transpose(spk, (0, 2, 1, 3)).reshape(STEPS, B, H2))
        mem_parts.append(np.transpose(mem, (0, 2, 1, 3)).reshape(STEPS, B, H2))
    spk2 = np.concatenate(spk_parts, axis=1).astype(np.float32)
    mem2 = np.concatenate(mem_parts, axis=1).astype(np.float32)
    return spk2, mem2


# revision 11
# speedup vs baseline: 1.1718x; 1.0526x over previous
"""Trainium2 Bass kernel for the AdaptiveSNN problem.

Strategy (pure data parallelism across 8 NeuronCores, batch 16384 -> 2048/core):
  - host: transpose x/W1 so the contraction dim (784) lands on SBUF partitions
  - cur1 = W1 @ x^T + b1 computed once per core in fp32 on the TensorEngine
    (layout [128 neurons, 2048 batch])
  - 25 sequential LIF steps; the whole membrane update
        m' = (beta*m + cur) - (m > 1)
    is ONE custom DVE instruction (registered below). Layer-1 spikes for the
    layer-2 matmul come from the ScalarEngine as Sign(m-1) in bf16 (+-1 exact).
  - layer-2 matmul per 128-batch chunk: stationary = sign chunk (bf16),
    moving = 0.5*W2^T split hi/lo in bf16 (exact to ~1e-7), plus a K=1
    fp32 ones x cc matmul that adds the constant 0.5*sum(W2)+b2 row.
    Output lands batch-major [128, 16*10] so layer-2 elementwise is cheap.
  - outputs DMA'd per step as [25, 128, 160] per core; host reassembles.
"""
import numpy as np
import ml_dtypes

import concourse.bacc as bacc
import concourse.mybir as mybir
import concourse.tile as tile
import concourse.dve_ops as dve_ops
from concourse.dve_spec import Spec, Src0, Src1, C0, C1
from concourse.dve_ops import DveOp
from concourse.bass_utils import run_bass_kernel_spmd

F32 = mybir.dt.float32
BF16 = mybir.dt.bfloat16

N_CORES = 8
B_FULL = 16384
B = B_FULL // N_CORES          # 2048 batch rows per core
D_IN = 784                     # 28*28
H1 = 128
H2 = 10
STEPS = 25
KT = 112                       # K-tile size: 784 = 7 * 112
NKT = D_IN // KT
NCHUNK = B // 128              # 16 batch chunks per core
THRESH = 1.0


def _register_lif():
    """Custom DVE op: out = ((in0*s0 + in1) - (in0 > s1)).

    Matches the reference's fp32 association order exactly:
    new_mem = (beta*mem + cur) - reset.
    """
    if "LIF_STEP_ANT" in dve_ops._SUB_OPCODE_FOR_NAME:
        return next(op for op in dve_ops.OPS if op.name == "LIF_STEP_ANT")
    op = DveOp(
        "LIF_STEP_ANT",
        Spec(
            body=(Src0 * C0 + Src1) - (Src0 > C1),
            reference=lambda in0, in1, s0, s1, imm2: (
                (in0 * s0 + in1) - (in0 > s1).astype(np.float32)
            ),
        ),
        subdim=False,
        uops_sha={"v3": "4d971942aba05d49", "v4": "da6677450a1cb1b9"},
    )
    dve_ops.OPS.append(op)
    dve_ops._SUB_OPCODE_FOR_NAME[op.name] = (
        dve_ops._CUSTOM_DVE_ROW_BASE + len(dve_ops.OPS) - 1
    )
    dve_ops.CUSTOM_DVE_SPECS[op.name] = op.spec
    return op


_GRAPH_CACHE = {}


def _build_graph(beta1: float, beta2: float):
    key = (beta1, beta2)
    if key in _GRAPH_CACHE:
        return _GRAPH_CACHE[key]
    LIF = _register_lif()
    Sign = mybir.ActivationFunctionType.Sign

    nc = bacc.Bacc("TRN2", target_bir_lowering=False, debug=False,
                   num_devices=N_CORES)

    xt_d = nc.dram_tensor("xt", [D_IN, B], F32, kind="ExternalInput").ap()
    w1t_d = nc.dram_tensor("w1t", [D_IN, H1], F32, kind="ExternalInput").ap()
    b1_d = nc.dram_tensor("b1", [H1, 1], F32, kind="ExternalInput").ap()
    w2h_d = nc.dram_tensor("w2h", [H1, H2], BF16, kind="ExternalInput").ap()
    w2l_d = nc.dram_tensor("w2l", [H1, H2], BF16, kind="ExternalInput").ap()
    cc_d = nc.dram_tensor("cc160", [128, NCHUNK * H2], F32, kind="ExternalInput").ap()

    out_spk = nc.dram_tensor("out_spk", [STEPS, 128, NCHUNK * H2], F32,
                             kind="ExternalOutput").ap()
    out_mem = nc.dram_tensor("out_mem", [STEPS, 128, NCHUNK * H2], F32,
                             kind="ExternalOutput").ap()

    with tile.TileContext(nc) as tc:
        with tc.tile_pool(name="const", bufs=1) as cpool, \
             tc.tile_pool(name="xin", bufs=1) as xpool, \
             tc.tile_pool(name="m1p", bufs=2) as m1pool, \
             tc.tile_pool(name="m2p", bufs=3) as m2pool, \
             tc.tile_pool(name="s2p", bufs=3) as s2pool, \
             tc.tile_pool(name="sgp", bufs=2) as sgpool, \
             tc.tile_pool(name="ps1", bufs=1, space="PSUM") as ps1pool, \
             tc.tile_pool(name="ps2", bufs=2, space="PSUM") as ps2pool:

            # preload the ACT function tables before anything else
            warm_t = cpool.tile([H1, 1], F32, tag="warm")
            nc.scalar.activation(warm_t[:], nc.const_aps.tensor(0.0, (H1, 1)),
                                 Sign, bias=0.0)
            # ---- load constants and input shard ----
            w1t_tiles = []
            xt_tiles = []
            for k in range(NKT):
                wt = cpool.tile([KT, H1], F32, tag=f"w1t{k}")
                nc.sync.dma_start(wt[:], w1t_d[k * KT:(k + 1) * KT, :])
                w1t_tiles.append(wt)
            for k in range(NKT):
                xt = xpool.tile([KT, B], F32, tag=f"xt{k}")
                half = B // 2
                nc.sync.dma_start(xt[:, :half], xt_d[k * KT:(k + 1) * KT, :half])
                nc.sync.dma_start(xt[:, half:], xt_d[k * KT:(k + 1) * KT, half:])
                xt_tiles.append(xt)
            b1_t = cpool.tile([H1, 1], F32, tag="b1")
            nc.sync.dma_start(b1_t[:], b1_d)
            w2h_t = cpool.tile([H1, H2], BF16, tag="w2h")
            nc.sync.dma_start(w2h_t[:], w2h_d)
            w2l_t = cpool.tile([H1, H2], BF16, tag="w2l")
            nc.sync.dma_start(w2l_t[:], w2l_d)
            cc_t = cpool.tile([128, NCHUNK * H2], F32, tag="cc160")
            nc.sync.dma_start(cc_t[:], cc_d)
            neg1_t = cpool.tile([H1, 1], F32, tag="neg1")
            nc.vector.memset(neg1_t[:], -1.0)

            # ---- cur1 = W1 @ x^T + b1 : [128 neurons, 2048 batch] ----
            cur1_t = cpool.tile([H1, B], F32, tag="cur1")
            NT = B // 512
            ps1_tiles = [ps1pool.tile([H1, 512], F32, tag=f"ps1_{nt}",
                                      name=f"ps1_{nt}") for nt in range(NT)]
            for k in range(NKT):
                for nt in range(NT):
                    nc.tensor.matmul(
                        ps1_tiles[nt][:], w1t_tiles[k][:],
                        xt_tiles[k][:, nt * 512:(nt + 1) * 512],
                        start=(k == 0), stop=(k == NKT - 1))
            for nt in range(NT):
                # psum -> sbuf with +b1 per-partition bias on the VectorEngine
                nc.vector.tensor_scalar(cur1_t[:, nt * 512:(nt + 1) * 512],
                                        ps1_tiles[nt][:], b1_t[:], None,
                                        mybir.AluOpType.add)

            # ---- states ----
            m1 = m1pool.tile([H1, B], F32, tag="m1")
            nc.vector.memset(m1[:], 0.0)
            m2 = m2pool.tile([128, NCHUNK * H2], F32, tag="m2")
            nc.vector.memset(m2[:], 0.0)
            r2 = s2pool.tile([128, NCHUNK * H2], F32, tag="s2")
            nc.vector.memset(r2[:], 0.0)

            # ---- time loop ----
            # Software-pipelined by one step: step t's layer-2 DVE work is
            # emitted AFTER step t+1's layer-1 LIF, so the DVE never stalls
            # waiting for the ScalarE(sign) -> TensorE(matmul) chain.
            def finish_l2(ps2, t, m2_cur, r_cur):
                # cur2 = psum + cc  (only step that must read PSUM -> DVE)
                cur2 = s2pool.tile([128, NCHUNK * H2], F32, tag="cur2")
                nc.vector.affine_then_add(cur2[:], ps2[:], cc_t[:], 1.0, 0.0)
                # membrane update on DVE (one fused op); spike on GpSimd
                m2n = m2pool.tile([128, NCHUNK * H2], F32, tag="m2")
                nc.vector._custom_dve(LIF, out=m2n[:], in0=m2_cur[:],
                                      in1=cur2[:], s0=beta2, s1=THRESH)
                g2 = s2pool.tile([128, NCHUNK * H2], F32, tag="g2")
                nc.scalar.activation(g2[:], m2n[:], Sign, bias=neg1_t[:])
                r_new = s2pool.tile([128, NCHUNK * H2], F32, tag="s2")
                nc.scalar.activation(r_new[:], g2[:],
                                     mybir.ActivationFunctionType.Relu)
                nc.sync.dma_start(out_mem[t], m2n[:])
                nc.sync.dma_start(out_spk[t], r_new[:])
                return m2n, r_new

            pend = None  # (psum2, t) of the step whose layer-2 is unfinished
            for t in range(STEPS):
                # layer-1 membrane update: one DVE instruction
                m1n = m1pool.tile([H1, B], F32, tag="m1")
                nc.vector._custom_dve(LIF, out=m1n[:], in0=m1[:], in1=cur1_t[:],
                                      s0=beta1, s1=THRESH)
                m1 = m1n
                if pend is not None:
                    m2, r2 = finish_l2(pend[0], pend[1], m2, r2)
                # spikes as signs (bf16, +-1) on the ScalarEngine
                sg = sgpool.tile([H1, B], BF16, tag="sg")
                nc.scalar.activation(sg[:], m1[:], Sign, bias=neg1_t[:])
                # layer-2: cur2 = cc + 0.5*W2@sg  (batch-major [128, 160])
                ps2 = ps2pool.tile([128, NCHUNK * H2], F32, tag="ps2")
                for c in range(NCHUNK):
                    o = ps2[:, c * H2:(c + 1) * H2]
                    sgc = sg[:, c * 128:(c + 1) * 128]
                    nc.tensor.matmul(o, sgc, w2h_t[:], start=True, stop=False)
                    nc.tensor.matmul(o, sgc, w2l_t[:], start=False, stop=True)
                pend = (ps2, t)
            m2, r2 = finish_l2(pend[0], pend[1], m2, r2)

    nc.compile()
    _GRAPH_CACHE[key] = nc
    return nc


def prepare_in_maps(x, W1, b1, W2, b2):
    x = np.asarray(x, dtype=np.float32)
    W1 = np.asarray(W1, dtype=np.float32)
    b1 = np.asarray(b1, dtype=np.float32)
    W2 = np.asarray(W2, dtype=np.float32)
    b2 = np.asarray(b2, dtype=np.float32)
    xf = x.reshape(B_FULL, D_IN)
    xT = np.ascontiguousarray(xf.T)                       # [784, 16384]
    W1T = np.ascontiguousarray(W1.T)                      # [784, 128]
    b1c = np.ascontiguousarray(b1.reshape(H1, 1))
    W2T_half = 0.5 * W2.T                                 # [128, 10]
    w2h = W2T_half.astype(ml_dtypes.bfloat16)
    w2l = (W2T_half - w2h.astype(np.float32)).astype(ml_dtypes.bfloat16)
    ccrow = (0.5 * W2.sum(axis=1) + b2).astype(np.float32)
    cc160 = np.ascontiguousarray(np.broadcast_to(np.tile(ccrow, NCHUNK), (128, NCHUNK * H2)).astype(np.float32))
    in_maps = []
    for i in range(N_CORES):
        shard = np.ascontiguousarray(xT[:, i * B:(i + 1) * B])
        in_maps.append({
            "xt": shard, "w1t": W1T, "b1": b1c,
            "w2h": w2h, "w2l": w2l, "cc160": cc160,
        })
    return in_maps


def kernel(x, W1, b1, W2, b2, beta1, beta2):
    bb1 = float(np.clip(np.float32(beta1), 0.0, 1.0))
    bb2 = float(np.clip(np.float32(beta2), 0.0, 1.0))
    in_maps = prepare_in_maps(x, W1, b1, W2, b2)
    nc = _build_graph(bb1, bb2)
    res = run_bass_kernel_spmd(nc, in_maps, list(range(N_CORES)), trace=False)

    spk_parts, mem_parts = [], []
    for i in range(N_CORES):
        r = res.results[i]
        # [25, 128, 16*10] -> [25, 2048, 10]: batch = chunk*128 + partition
        spk = r["out_spk"].reshape(STEPS, 128, NCHUNK, H2)
        mem = r["out_mem"].reshape(STEPS, 128, NCHUNK, H2)
        spk_parts.append(np.transpose(spk, (0, 2, 1, 3)).reshape(STEPS, B, H2))
        mem_parts.append(np.transpose(mem, (0, 2, 1, 3)).reshape(STEPS, B, H2))
    spk2 = np.concatenate(spk_parts, axis=1).astype(np.float32)
    mem2 = np.concatenate(mem_parts, axis=1).astype(np.float32)
    return spk2, mem2


# revision 12
# speedup vs baseline: 1.1920x; 1.0172x over previous
"""Trainium2 Bass kernel for the AdaptiveSNN problem.

Strategy (pure data parallelism across 8 NeuronCores, batch 16384 -> 2048/core):
  - host: transpose x/W1 so the contraction dim (784) lands on SBUF partitions
  - cur1 = W1 @ x^T + b1 computed once per core in fp32 on the TensorEngine
    (layout [128 neurons, 2048 batch])
  - 25 sequential LIF steps; the whole membrane update
        m' = (beta*m + cur) - (m > 1)
    is ONE custom DVE instruction (registered below). Layer-1 spikes for the
    layer-2 matmul come from the ScalarEngine as Sign(m-1) in bf16 (+-1 exact).
  - layer-2 matmul per 128-batch chunk: stationary = sign chunk (bf16),
    moving = 0.5*W2^T split hi/lo in bf16 (exact to ~1e-7), plus a K=1
    fp32 ones x cc matmul that adds the constant 0.5*sum(W2)+b2 row.
    Output lands batch-major [128, 16*10] so layer-2 elementwise is cheap.
  - outputs DMA'd per step as [25, 128, 160] per core; host reassembles.
"""
import numpy as np
import ml_dtypes

import concourse.bacc as bacc
import concourse.mybir as mybir
import concourse.tile as tile
from concourse.tile import add_dep_helper
import concourse.dve_ops as dve_ops
from concourse.dve_spec import Spec, Src0, Src1, C0, C1
from concourse.dve_ops import DveOp
from concourse.bass_utils import run_bass_kernel_spmd

F32 = mybir.dt.float32
BF16 = mybir.dt.bfloat16

N_CORES = 8
B_FULL = 16384
B = B_FULL // N_CORES          # 2048 batch rows per core
D_IN = 784                     # 28*28
H1 = 128
H2 = 10
STEPS = 25
KT = 112                       # K-tile size: 784 = 7 * 112
NKT = D_IN // KT
NCHUNK = B // 128              # 16 batch chunks per core
THRESH = 1.0


def _register_lif():
    """Custom DVE op: out = ((in0*s0 + in1) - (in0 > s1)).

    Matches the reference's fp32 association order exactly:
    new_mem = (beta*mem + cur) - reset.
    """
    if "LIF_STEP_ANT" in dve_ops._SUB_OPCODE_FOR_NAME:
        return next(op for op in dve_ops.OPS if op.name == "LIF_STEP_ANT")
    op = DveOp(
        "LIF_STEP_ANT",
        Spec(
            body=(Src0 * C0 + Src1) - (Src0 > C1),
            reference=lambda in0, in1, s0, s1, imm2: (
                (in0 * s0 + in1) - (in0 > s1).astype(np.float32)
            ),
        ),
        subdim=False,
        uops_sha={"v3": "4d971942aba05d49", "v4": "da6677450a1cb1b9"},
    )
    dve_ops.OPS.append(op)
    dve_ops._SUB_OPCODE_FOR_NAME[op.name] = (
        dve_ops._CUSTOM_DVE_ROW_BASE + len(dve_ops.OPS) - 1
    )
    dve_ops.CUSTOM_DVE_SPECS[op.name] = op.spec
    return op


_GRAPH_CACHE = {}


def _build_graph(beta1: float, beta2: float):
    key = (beta1, beta2)
    if key in _GRAPH_CACHE:
        return _GRAPH_CACHE[key]
    LIF = _register_lif()
    Sign = mybir.ActivationFunctionType.Sign

    nc = bacc.Bacc("TRN2", target_bir_lowering=False, debug=False,
                   num_devices=N_CORES)

    xt_d = nc.dram_tensor("xt", [D_IN, B], F32, kind="ExternalInput").ap()
    w1t_d = nc.dram_tensor("w1t", [D_IN, H1], F32, kind="ExternalInput").ap()
    b1_d = nc.dram_tensor("b1", [H1, 1], F32, kind="ExternalInput").ap()
    w2h_d = nc.dram_tensor("w2h", [H1, H2], BF16, kind="ExternalInput").ap()
    w2l_d = nc.dram_tensor("w2l", [H1, H2], BF16, kind="ExternalInput").ap()
    cc_d = nc.dram_tensor("cc160", [128, NCHUNK * H2], F32, kind="ExternalInput").ap()

    out_spk = nc.dram_tensor("out_spk", [STEPS, 128, NCHUNK * H2], F32,
                             kind="ExternalOutput").ap()
    out_mem = nc.dram_tensor("out_mem", [STEPS, 128, NCHUNK * H2], F32,
                             kind="ExternalOutput").ap()

    with tile.TileContext(nc) as tc:
        with tc.tile_pool(name="const", bufs=1) as cpool, \
             tc.tile_pool(name="xin", bufs=1) as xpool, \
             tc.tile_pool(name="m1p", bufs=2) as m1pool, \
             tc.tile_pool(name="m2p", bufs=3) as m2pool, \
             tc.tile_pool(name="s2p", bufs=3) as s2pool, \
             tc.tile_pool(name="sgp", bufs=2) as sgpool, \
             tc.tile_pool(name="ps1", bufs=1, space="PSUM") as ps1pool, \
             tc.tile_pool(name="ps2", bufs=2, space="PSUM") as ps2pool:

            # preload the ACT function tables before anything else
            warm_t = cpool.tile([H1, 1], F32, tag="warm")
            nc.scalar.activation(warm_t[:], nc.const_aps.tensor(0.0, (H1, 1)),
                                 Sign, bias=0.0)
            # ---- load constants and input shard ----
            w1t_tiles = []
            xt_tiles = []
            for k in range(NKT):
                wt = cpool.tile([KT, H1], F32, tag=f"w1t{k}")
                nc.sync.dma_start(wt[:], w1t_d[k * KT:(k + 1) * KT, :])
                w1t_tiles.append(wt)
            # Chain xt DMAs with a 2-K in-flight window: the HW queue
            # round-robins packets across all outstanding transfers, which
            # would make every K-slice arrive simultaneously at the end;
            # chaining gives sequential arrival so matmuls pipeline behind.
            xt_dmas = []
            for k in range(NKT):
                xt = xpool.tile([KT, B], F32, tag=f"xt{k}")
                half = B // 2
                d0 = nc.sync.dma_start(xt[:, :half],
                                       xt_d[k * KT:(k + 1) * KT, :half])
                d1 = nc.sync.dma_start(xt[:, half:],
                                       xt_d[k * KT:(k + 1) * KT, half:])
                if k >= 2:
                    prev = xt_dmas[k - 2]
                    add_dep_helper(d0.ins, prev[0].ins, sync=True,
                                   reason="stage xt arrival")
                    add_dep_helper(d1.ins, prev[1].ins, sync=True,
                                   reason="stage xt arrival")
                xt_dmas.append((d0, d1))
                xt_tiles.append(xt)
            b1_t = cpool.tile([H1, 1], F32, tag="b1")
            nc.sync.dma_start(b1_t[:], b1_d)
            w2h_t = cpool.tile([H1, H2], BF16, tag="w2h")
            nc.sync.dma_start(w2h_t[:], w2h_d)
            w2l_t = cpool.tile([H1, H2], BF16, tag="w2l")
            nc.sync.dma_start(w2l_t[:], w2l_d)
            cc_t = cpool.tile([128, NCHUNK * H2], F32, tag="cc160")
            nc.sync.dma_start(cc_t[:], cc_d)
            neg1_t = cpool.tile([H1, 1], F32, tag="neg1")
            nc.vector.memset(neg1_t[:], -1.0)

            # ---- cur1 = W1 @ x^T + b1 : [128 neurons, 2048 batch] ----
            cur1_t = cpool.tile([H1, B], F32, tag="cur1")
            NT = B // 512
            ps1_tiles = [ps1pool.tile([H1, 512], F32, tag=f"ps1_{nt}",
                                      name=f"ps1_{nt}") for nt in range(NT)]
            for k in range(NKT):
                for nt in range(NT):
                    nc.tensor.matmul(
                        ps1_tiles[nt][:], w1t_tiles[k][:],
                        xt_tiles[k][:, nt * 512:(nt + 1) * 512],
                        start=(k == 0), stop=(k == NKT - 1))
            for nt in range(NT):
                # psum -> sbuf with +b1 per-partition bias on the VectorEngine
                nc.vector.tensor_scalar(cur1_t[:, nt * 512:(nt + 1) * 512],
                                        ps1_tiles[nt][:], b1_t[:], None,
                                        mybir.AluOpType.add)

            # ---- states ----
            m1 = m1pool.tile([H1, B], F32, tag="m1")
            nc.vector.memset(m1[:], 0.0)
            m2 = m2pool.tile([128, NCHUNK * H2], F32, tag="m2")
            nc.vector.memset(m2[:], 0.0)
            r2 = s2pool.tile([128, NCHUNK * H2], F32, tag="s2")
            nc.vector.memset(r2[:], 0.0)

            # ---- time loop ----
            # Software-pipelined by one step: step t's layer-2 DVE work is
            # emitted AFTER step t+1's layer-1 LIF, so the DVE never stalls
            # waiting for the ScalarE(sign) -> TensorE(matmul) chain.
            def finish_l2(ps2, t, m2_cur, r_cur):
                # cur2 = psum + cc  (only step that must read PSUM -> DVE)
                cur2 = s2pool.tile([128, NCHUNK * H2], F32, tag="cur2")
                nc.vector.affine_then_add(cur2[:], ps2[:], cc_t[:], 1.0, 0.0)
                # membrane update on DVE (one fused op); spike on GpSimd
                m2n = m2pool.tile([128, NCHUNK * H2], F32, tag="m2")
                nc.vector._custom_dve(LIF, out=m2n[:], in0=m2_cur[:],
                                      in1=cur2[:], s0=beta2, s1=THRESH)
                g2 = s2pool.tile([128, NCHUNK * H2], F32, tag="g2")
                nc.scalar.activation(g2[:], m2n[:], Sign, bias=neg1_t[:])
                r_new = s2pool.tile([128, NCHUNK * H2], F32, tag="s2")
                nc.scalar.activation(r_new[:], g2[:],
                                     mybir.ActivationFunctionType.Relu)
                nc.sync.dma_start(out_mem[t], m2n[:])
                nc.sync.dma_start(out_spk[t], r_new[:])
                return m2n, r_new

            pend = None  # (psum2, t) of the step whose layer-2 is unfinished
            for t in range(STEPS):
                # layer-1 membrane update: one DVE instruction
                m1n = m1pool.tile([H1, B], F32, tag="m1")
                nc.vector._custom_dve(LIF, out=m1n[:], in0=m1[:], in1=cur1_t[:],
                                      s0=beta1, s1=THRESH)
                m1 = m1n
                if pend is not None:
                    m2, r2 = finish_l2(pend[0], pend[1], m2, r2)
                # spikes as signs (bf16, +-1) on the ScalarEngine
                sg = sgpool.tile([H1, B], BF16, tag="sg")
                nc.scalar.activation(sg[:], m1[:], Sign, bias=neg1_t[:])
                # layer-2: cur2 = cc + 0.5*W2@sg  (batch-major [128, 160])
                ps2 = ps2pool.tile([128, NCHUNK * H2], F32, tag="ps2")
                for c in range(NCHUNK):
                    o = ps2[:, c * H2:(c + 1) * H2]
                    sgc = sg[:, c * 128:(c + 1) * 128]
                    nc.tensor.matmul(o, sgc, w2h_t[:], start=True, stop=False)
                    nc.tensor.matmul(o, sgc, w2l_t[:], start=False, stop=True)
                pend = (ps2, t)
            m2, r2 = finish_l2(pend[0], pend[1], m2, r2)

    nc.compile()
    _GRAPH_CACHE[key] = nc
    return nc


def prepare_in_maps(x, W1, b1, W2, b2):
    x = np.asarray(x, dtype=np.float32)
    W1 = np.asarray(W1, dtype=np.float32)
    b1 = np.asarray(b1, dtype=np.float32)
    W2 = np.asarray(W2, dtype=np.float32)
    b2 = np.asarray(b2, dtype=np.float32)
    xf = x.reshape(B_FULL, D_IN)
    xT = np.ascontiguousarray(xf.T)                       # [784, 16384]
    W1T = np.ascontiguousarray(W1.T)                      # [784, 128]
    b1c = np.ascontiguousarray(b1.reshape(H1, 1))
    W2T_half = 0.5 * W2.T                                 # [128, 10]
    w2h = W2T_half.astype(ml_dtypes.bfloat16)
    w2l = (W2T_half - w2h.astype(np.float32)).astype(ml_dtypes.bfloat16)
    ccrow = (0.5 * W2.sum(axis=1) + b2).astype(np.float32)
    cc160 = np.ascontiguousarray(np.broadcast_to(np.tile(ccrow, NCHUNK), (128, NCHUNK * H2)).astype(np.float32))
    in_maps = []
    for i in range(N_CORES):
        shard = np.ascontiguousarray(xT[:, i * B:(i + 1) * B])
        in_maps.append({
            "xt": shard, "w1t": W1T, "b1": b1c,
            "w2h": w2h, "w2l": w2l, "cc160": cc160,
        })
    return in_maps


def kernel(x, W1, b1, W2, b2, beta1, beta2):
    bb1 = float(np.clip(np.float32(beta1), 0.0, 1.0))
    bb2 = float(np.clip(np.float32(beta2), 0.0, 1.0))
    in_maps = prepare_in_maps(x, W1, b1, W2, b2)
    nc = _build_graph(bb1, bb2)
    res = run_bass_kernel_spmd(nc, in_maps, list(range(N_CORES)), trace=False)

    spk_parts, mem_parts = [], []
    for i in range(N_CORES):
        r = res.results[i]
        # [25, 128, 16*10] -> [25, 2048, 10]: batch = chunk*128 + partition
        spk = r["out_spk"].reshape(STEPS, 128, NCHUNK, H2)
        mem = r["out_mem"].reshape(STEPS, 128, NCHUNK, H2)
        spk_parts.append(np.transpose(spk, (0, 2, 1, 3)).reshape(STEPS, B, H2))
        mem_parts.append(np.transpose(mem, (0, 2, 1, 3)).reshape(STEPS, B, H2))
    spk2 = np.concatenate(spk_parts, axis=1).astype(np.float32)
    mem2 = np.concatenate(mem_parts, axis=1).astype(np.float32)
    return spk2, mem2
